# revision 78
# baseline (speedup 1.0000x reference)
"""Self-contained Trainium2 Bass kernel for MBert self-attention.

Problem (hardcoded): B=4, T=2048, C=768, H=12 heads, D=64.
  q = X @ Wq.T + bq ; k = X @ Wk.T + bk ; v = X @ Wv.T + bv   (per batch)
  scores = q k^T / sqrt(D) + mask_bias ; probs = softmax(scores)
  out = probs @ v                                              (per head)

Sharding over 8 NeuronCores: data-parallel on B (4) x tensor-parallel on
heads (12 -> two groups of 6).  Core c handles batch c//2 and heads
6*(c%2) .. 6*(c%2)+5.  Each core computes its full [T, 384] output slice
locally; host concatenates (no device collectives needed).

Design (all matmuls bf16, fp32 PSUM accumulation):
  - Host pre-casts X to bf16 and passes W^T slices [C, O] bf16 (weight
    layout prep), biases fp32, and the mask factor f = exp(-1e4*(1-m)).
  - X^T loaded via HW DMA-transpose instructions in three t-tranches
    (no PE transposes); weight/bias DMA order matches consumer order.
  - Q^T/K^T projections produce [o, t] bf16 tiles (bias on DVE or
    ScalarE); V in natural [t, o] layout with a ones column (65 cols) so
    the attention-V matmul also emits the softmax denominator; V rows
    pre-scaled by f.
  - Attention per head pair / 512-wide q group / 128-wide k chunk:
    S^T[kchunk, q] two matmuls into one [128,1024] PSUM tile from a
    3-deep rotation (shared with the projection accumulators -- 6 banks
    + 2 ctx banks = all 8; an accumulation region owns its whole bank
    since start=True zeroes bank-wide).
  - exp split per group between ScalarE (exact, scale=1/8 fused) and
    DVE (Schraudolph int16 map directly into the bf16 pT bits); the
    split tracks which engine would otherwise idle.
  - AV re-oriented: ctx[qtile, 65] = sum_chunks pT_chunk[:, qslice].T @
    [V|1]_chunk -- 65-row matmuls with full 128-partition output, half
    the PE rows of the [65, q] orientation, and no epilogue transposes.
    One ctx slot accumulates at a time per bank (slot-sequential).
  - Normalize: DVE copies ctx to SBUF; GpSimd normalize_recip divides
    by the denominator column into the fp32 staging buffer; DMA out.
  - Emission is hook-scheduled so the in-order PE queue never blocks on
    a late DMA: V tiles and remaining projections thread through the
    attention chunk stream where their inputs land.
"""

import collections

import numpy as np

B, T, C = 4, 2048, 768
H, D = 12, 64
NCORES = 8
HLOC = 6              # heads per core
O = HLOC * D          # 384 output cols per core
NPAIR = HLOC // 2     # 3 head pairs
CCH = C // 128        # 6 contraction chunks for projections
TT = T // 128         # 16 t tiles
QG = 512              # q-group width
NG = T // QG          # 4 q groups
KCH = T // 128        # 16 k chunks

_CACHE = {}


def _build_nc():
    if "nc" in _CACHE:
        return _CACHE["nc"]

    from contextlib import ExitStack

    import concourse.bass as bass
    import concourse.tile as tile
    from concourse import bacc, mybir

    f32 = mybir.dt.float32
    bf16 = mybir.dt.bfloat16
    i16 = mybir.dt.int16
    EXP = mybir.ActivationFunctionType.Exp
    MULT = mybir.AluOpType.mult
    ADD = mybir.AluOpType.add

    # Schraudolph-style exp for offloaded chunks: the bf16 bit pattern of
    # exp(s/8) is approximated by the int16 affine map trunc(s*A2 + B2)
    # (sign|exp|mantissa of bf16 viewed as an integer is linear in log2).
    # Constants fit numerically for min rms (1.8%) over the score range.
    A2 = 0.125 * 128.0 / 0.6931471805599453
    B2 = 16249.15

    nc = bacc.Bacc("TRN2", target_bir_lowering=False, debug=False,
                   num_devices=NCORES)

    x_d = nc.dram_tensor("x", [T, C], bf16, kind="ExternalInput").ap()
    wt_d = {}
    b_d = {}
    for nm in ("q", "k", "v"):
        wt_d[nm] = nc.dram_tensor(f"wt{nm}", [C, O], bf16,
                                  kind="ExternalInput").ap()
        shp = [O] if nm == "v" else [128, O // 128]
        b_d[nm] = nc.dram_tensor(f"b{nm}", shp, f32,
                                 kind="ExternalInput").ap()
    f_d = nc.dram_tensor("fmask", [128, KCH], f32, kind="ExternalInput").ap()
    o_d = nc.dram_tensor("out", [T, O], f32, kind="ExternalOutput").ap()

    with tile.TileContext(nc) as tc, ExitStack() as ctx:
        # ---------------- persistent pools ----------------
        const = ctx.enter_context(tc.tile_pool(name="const", bufs=1))
        xT_pool = ctx.enter_context(tc.tile_pool(name="xT", bufs=1))
        wT_pool = ctx.enter_context(tc.tile_pool(name="wT", bufs=1))
        qkT_pool = ctx.enter_context(tc.tile_pool(name="qkT", bufs=1))
        v_pool = ctx.enter_context(tc.tile_pool(name="v", bufs=1))
        ost_pool = ctx.enter_context(tc.tile_pool(name="ostage", bufs=1))

        # biases for q/k in [o mod 128, o // 128] layout (per-partition use)
        bqk_t = {nm: const.tile([128, O // 128], f32, name=f"bias_{nm}")
                 for nm in ("q", "k")}
        # v bias broadcast to all partitions (varies along free dim there)
        bv_bc = const.tile([128, O], f32)
        # mask factor f[t] in [t mod 128, t // 128] layout
        f_t = const.tile([128, KCH], f32)

        xT = xT_pool.tile([128, CCH, T], bf16)          # X^T  [c, t]
        wT = {nm: wT_pool.tile([128, CCH, O], bf16, name=f"wT_{nm}")
              for nm in ("q", "k", "v")}               # W^T  [c, o]
        qT = qkT_pool.tile([128, O // 128, T], bf16, name="qT")   # Q^T [o, t]
        kT = qkT_pool.tile([128, O // 128, T], bf16, name="kT")   # K^T [o, t]
        v_sb = v_pool.tile([128, KCH, HLOC, D + 1], bf16)         # V|1 [k, h, d]
        ostage = ost_pool.tile([128, TT, O], f32)      # output rows staging

        # ones column for the denominator trick (scaled by f below)
        nc.vector.memset(v_sb[:, :, :, D], 1.0)
        # zero tile for PE warm-up matmuls (see below)
        zz = const.tile([128, QG], bf16, name="zz")
        nc.vector.memset(zz[:], 0.0)

        # ---------------- loads ----------------
        # HWDGE/DMA are serial devices and the PE queue is in-order, so the
        # arrival order must match the emission order's needs: Wq/Wk, then
        # the X^T t-tranches in the order the k-projections consume them
        # (t0 for qk-g0 and S^T chunks 0-3, t1 for k-g1, t23 for k-g2/3),
        # with Wv slotted in for the early V tiles.
        for nm in ("q", "k"):
            nc.sync.dma_start(
                wT[nm][:, :, 0:128],
                wt_d[nm][:, 0:128].rearrange("(cc p) o -> p cc o", p=128))
        for nm in ("q", "k"):
            nc.sync.dma_start(bqk_t[nm][:], b_d[nm][:])
        for cc in range(CCH):
            nc.sync.dma_start_transpose(
                xT[:, cc, 0:QG], x_d[0:QG, 128 * cc:128 * (cc + 1)])
        nc.sync.dma_start(wT["v"][:],
                          wt_d["v"].rearrange("(cc p) o -> p cc o", p=128))
        nc.sync.dma_start(bv_bc[:], b_d["v"].unsqueeze(0).broadcast_to([128, O]))
        nc.sync.dma_start(f_t[:], f_d[:])
        for cc in range(CCH):
            nc.sync.dma_start_transpose(
                xT[:, cc, QG:2 * QG], x_d[QG:2 * QG, 128 * cc:128 * (cc + 1)])
        for cc in range(CCH):
            nc.sync.dma_start_transpose(
                xT[:, cc, 2 * QG:T], x_d[2 * QG:T, 128 * cc:128 * (cc + 1)])
        for nm in ("q", "k"):
            nc.sync.dma_start(
                wT[nm][:, :, 128:O],
                wt_d[nm][:, 128:O].rearrange("(cc p) o -> p cc o", p=128))

        # ---------------- attention + projections ----------------
        # One PSUM pool (3 x [128, 1024] = 6 banks) is shared by the S^T
        # tiles AND the projection/V accumulators (as sub-views), so the
        # score pipeline runs 3 deep; pctx takes the remaining 2 banks.
        stage_d = ExitStack()
        pst_pool = stage_d.enter_context(
            tc.tile_pool(name="pst", bufs=3, space="PSUM"))
        pT_pool = stage_d.enter_context(tc.tile_pool(name="pT", bufs=2))
        nrm_pool = stage_d.enter_context(tc.tile_pool(name="nrm", bufs=4))
        # V projections get their own PSUM banks while the ctx banks are
        # still idle (first AV slot is emitted at group 1, chunk 8): pv
        # opens first and closes just before pctx opens in its place.
        pv_stack = ExitStack()
        pv_pool = pv_stack.enter_context(
            tc.tile_pool(name="pv", bufs=2, space="PSUM"))
        pctx_stack = ExitStack()
        pools = {}

        def open_pctx():
            pv_stack.close()
            pools["pctx"] = pctx_stack.enter_context(
                tc.tile_pool(name="pctx", bufs=2, space="PSUM"))

        def _ps_tile():
            return pst_pool.tile([128, 2 * QG], f32, name="st_ps", tag="st")

        qk_alt = [0]

        def emit_qk_g(j, g, names, alt=False, pad=0):
            """Project Q^T / K^T for (o-tile j, q-group g). The bias add +
            bf16 downcast goes to DVE, or alternates DVE/ScalarE when both
            are loaded (alt=True). pad>0 interleaves warm-up matmuls after
            each contraction chunk to bridge staggered X^T DMA arrivals
            (a PE idle gap resets the p-state ramp)."""
            for nm in names:
                dst = qT if nm == "q" else kT
                ps = _ps_tile()
                for ci in range(CCH):
                    nc.tensor.matmul(
                        ps[:, 0:QG],
                        lhsT=wT[nm][:, ci, 128 * j:128 * (j + 1)],
                        rhs=xT[:, ci, QG * g:QG * (g + 1)],
                        start=(ci == 0), stop=(ci == CCH - 1))
                    for _ in range(pad if ci < CCH - 1 else 0):
                        nc.tensor.matmul(pad_ps[:, 0:O], lhsT=zz[:, 0:128],
                                         rhs=zz[:, 0:O])
                qk_alt[0] += 1
                if alt and qk_alt[0] % 2 == 0:
                    nc.scalar.add(dst[:, j, QG * g:QG * (g + 1)], ps[:, 0:QG],
                                  bqk_t[nm][:, j:j + 1])
                else:
                    nc.vector.tensor_scalar_add(
                        dst[:, j, QG * g:QG * (g + 1)], ps[:, 0:QG],
                        bqk_t[nm][:, j:j + 1])

        def emit_v_tile(i):
            """V projection for t-tile i into the [k, h, d|1] slab."""
            ps = pv_pool.tile([128, O], f32, name="v_ps", tag="v")
            for ci in range(CCH):
                nc.tensor.matmul(
                    ps[:, 0:O],
                    lhsT=xT[:, ci, 128 * i:128 * (i + 1)],
                    rhs=wT["v"][:, ci, :],
                    start=(ci == 0), stop=(ci == CCH - 1))
            nc.vector.tensor_add(
                v_sb[:, i, :, 0:D],
                ps[:, 0:O].rearrange("p (h d) -> p h d", h=HLOC),
                bv_bc.rearrange("p (h d) -> p h d", h=HLOC))
            nc.vector.tensor_scalar_mul(v_sb[:, i], v_sb[:, i], f_t[:, i:i + 1])

        # PE warm-up: the cost model halves matmul throughput until the PE
        # has been continuously busy ~3us, and any idle gap resets the
        # ramp. The real work can't start until the first weight/X^T DMAs
        # land (~5us), so burn that idle window with zero matmuls (into a
        # scratch bank of the still-idle pv pool) and bridge the staggered
        # X^T arrivals inside the first projection the same way.
        pad_ps = pv_pool.tile([128, O], f32, name="v_ps", tag="v")
        for _ in range(14):
            nc.tensor.matmul(pad_ps[:, 0:O], lhsT=zz[:, 0:128],
                             rhs=zz[:, 0:O])

        emit_qk_g(0, 0, ("q",))
        # k-g0 in two column pieces: S^T chunk 0 only reads kT[:, 0, 0:128],
        # so a narrow first piece (and its cheap bias copy) un-gates the
        # first exp ~2us earlier than the full 512-wide projection would.
        for (c0, c1) in ((0, 128), (128, QG)):
            ps = _ps_tile()
            for ci in range(CCH):
                nc.tensor.matmul(
                    ps[:, 0:c1 - c0],
                    lhsT=wT["k"][:, ci, 0:128],
                    rhs=xT[:, ci, c0:c1],
                    start=(ci == 0), stop=(ci == CCH - 1))
            nc.vector.tensor_scalar_add(
                kT[:, 0, c0:c1], ps[:, 0:c1 - c0], bqk_t["k"][:, 0:1])

        groups = [(p, g) for p in range(NPAIR) for g in range(NG)]
        state = {}  # (p, g) -> pT_tile

        def exp_engine(gi, i):
            """Exp consumer per (group index, chunk). Early groups are
            PE-bound by the interleaved projections, so the exact ScalarE
            exp covers (almost) everything; in steady-state groups the
            Schraudolph int16 map on DVE picks up enough chunks for
            ScalarE to match the PE cadence (only ScalarE and DVE can
            read the PSUM scores)."""
            if gi == 0:
                return "act"        # fat PE window: ScalarE keeps up alone
            if gi <= 4:
                return "dve" if i in (3, 7, 11, 15) else "act"
            return "dve" if i in (1, 4, 6, 9, 12, 14) else "act"

        def emit_st_exp(gi, p, g):
            """S^T + exp for all 16 chunks of (p, g)."""
            pTt = pT_pool.tile([128, KCH, 2 * QG], bf16, name="pT", tag="pT")
            state[(p, g)] = pTt
            q0 = QG * g
            for i in range(KCH):
                st = _ps_tile()
                nc.tensor.matmul(
                    st[:, 0:QG],
                    lhsT=kT[0:64, p, 128 * i:128 * (i + 1)],
                    rhs=qT[0:64, p, q0:q0 + QG])
                nc.tensor.matmul(
                    st[:, QG:2 * QG],
                    lhsT=kT[64:128, p, 128 * i:128 * (i + 1)],
                    rhs=qT[64:128, p, q0:q0 + QG])
                eng = exp_engine(gi, i)
                if eng == "act":
                    nc.scalar.activation(pTt[:, i, :], st[:], EXP, scale=0.125)
                elif eng == "dve":
                    nc.vector.tensor_scalar(
                        pTt.bitcast(i16)[:, i, :], st[:], A2, B2, MULT, ADD)
                else:
                    nc.scalar.activation(pTt[:, i, 0:QG], st[:, 0:QG],
                                         EXP, scale=0.125)
                    nc.vector.tensor_scalar(
                        pTt.bitcast(i16)[:, i, QG:2 * QG], st[:, QG:2 * QG],
                        A2, B2, MULT, ADD)
                yield i

        def emit_av_slot(p, g, s, tail=False):
            """One ctx slot (qtile, head) of group (p, g): 16 accumulation
            matmuls into a bank-sized PSUM tile, then normalize into ostage
            (accumulation regions may not share a PSUM bank: a start=True
            matmul marks the whole 2KB bank pending-zero). In the tail the
            S^T pool is idle, so half the slots borrow its banks to run
            all accumulations back-to-back without WAR stalls."""
            pTt = state[(p, g)]
            qt, hh = divmod(s, 2)
            h = 2 * p + hh
            tt = 4 * g + qt
            if tail and s % 2 == 0:
                ctx = _ps_tile()[:, 0:D + 1]
            else:
                ctx = pools["pctx"].tile([128, D + 1], f32, name="ctx",
                                         tag="ctx")
            for i in range(KCH):
                nc.tensor.matmul(
                    ctx[:],
                    lhsT=pTt[:, i, QG * hh + 128 * qt:QG * hh + 128 * (qt + 1)],
                    rhs=v_sb[:, i, h, :],
                    start=(i == 0), stop=(i == KCH - 1))
            # DVE stages the 65-col ctx to SBUF; the otherwise-idle GpSimd
            # engine does the fused divide-by-denominator.
            ctx_sb = nrm_pool.tile([128, D + 1], f32, name="ctx_sb", tag="csb")
            nc.vector.tensor_copy(ctx_sb[:], ctx[:])
            nc.gpsimd.normalize_recip(
                ostage[:, tt, D * h:D * (h + 1)], ctx_sb[:, 0:D],
                ctx_sb[:, D:D + 1])
            if p == NPAIR - 1 and hh == 1:
                nc.sync.dma_start(o_d[128 * tt:128 * (tt + 1), :],
                                  ostage[:, tt, :])
            if s == 7:
                state.pop((p, g))

        # Per-(group, chunk) emission hooks: projections and V tiles are
        # threaded through the attention chunk stream exactly where their
        # DMA/compute dependencies land, so the in-order PE queue never
        # blocks on a late input.
        hooks = collections.defaultdict(list)
        for i in range(4):
            hooks[(0, i)].append(lambda i=i: emit_v_tile(i))
        hooks[(0, 3)].append(lambda: emit_qk_g(0, 1, ("k",)))
        hooks[(0, 7)].append(lambda: emit_qk_g(0, 2, ("k",)))
        hooks[(0, 11)].append(lambda: emit_qk_g(0, 3, ("k",)))
        hooks[(0, 13)].append(lambda: emit_qk_g(0, 1, ("q",)))
        for i in range(4, TT):          # V tiles 4..15, two per chunk
            hooks[(1, (i - 4) // 2)].append(lambda i=i: emit_v_tile(i))
        hooks[(1, 7)].append(open_pctx)
        hooks[(1, 7)].append(lambda: emit_qk_g(0, 2, ("q",)))
        for gg in range(NG):
            hooks[(2, 2 * gg + 1)].append(
                lambda gg=gg: emit_qk_g(1, gg, ("k",), alt=True))
        hooks[(2, 13)].append(lambda: emit_qk_g(0, 3, ("q",), alt=True))
        for gg in range(NG):
            hooks[(3, 2 * gg + 1)].append(
                lambda gg=gg: emit_qk_g(1, gg, ("q",), alt=True))
            hooks[(4, 2 * gg + 1)].append(
                lambda gg=gg: emit_qk_g(2, gg, ("k",), alt=True))
            hooks[(5, 2 * gg + 1)].append(
                lambda gg=gg: emit_qk_g(2, gg, ("q",), alt=True))

        def av_sched(gi, i):
            """Which ctx slots of the PREVIOUS group to run at chunk i."""
            if gi == 1:
                return [i - 8] if i >= 8 else []
            return [i // 2] if i % 2 == 1 else []

        prev = None
        for gi, (p, g) in enumerate(groups):
            for fn_ in hooks[(gi, -1)]:
                fn_()
            gen = emit_st_exp(gi, p, g)
            for i in gen:
                for fn_ in hooks[(gi, i)]:
                    fn_()
                if prev is not None:
                    for s in av_sched(gi, i):
                        emit_av_slot(*prev, s)
            prev = (p, g)
        for s in range(8):
            emit_av_slot(*prev, s, tail=True)

        pctx_stack.close()
        stage_d.close()

    nc.compile()
    _CACHE["nc"] = nc
    return nc


def _in_maps(inputs):
    import ml_dtypes
    bf16 = ml_dtypes.bfloat16
    hs = np.asarray(inputs["hidden_states"], dtype=np.float32)
    mask = np.asarray(inputs["attention_mask"], dtype=np.float32)
    W = {nm: np.asarray(inputs["W" + nm], dtype=np.float32)
         for nm in ("q", "k", "v")}
    bias = {nm: np.asarray(inputs["b" + nm], dtype=np.float32)
            for nm in ("q", "k", "v")}
    f = np.exp((mask.astype(np.float64) - 1.0) * 10000.0).astype(np.float32)
    x_bf = hs.astype(bf16)
    wt_bf = {nm: {} for nm in ("q", "k", "v")}
    for nm in ("q", "k", "v"):
        for hh in range(2):
            o0 = hh * O
            wt_bf[nm][hh] = np.ascontiguousarray(
                W[nm][o0:o0 + O].T.astype(bf16))
    maps = []
    for c in range(NCORES):
        b, hh = divmod(c, 2)
        o0 = hh * O
        # fmask/bq/bk pre-laid-out as [partition, free] so the device DMA
        # is a contiguous load instead of a descriptor-bound 4B rearrange
        m = {"x": np.ascontiguousarray(x_bf[b]),
             "fmask": np.ascontiguousarray(
                 f[b].reshape(KCH, 128).T.astype(np.float32))}
        for nm in ("q", "k", "v"):
            m["wt" + nm] = wt_bf[nm][hh]
            bslice = bias[nm][o0:o0 + O]
            if nm == "v":
                m["b" + nm] = np.ascontiguousarray(bslice)
            else:
                m["b" + nm] = np.ascontiguousarray(
                    bslice.reshape(O // 128, 128).T)
        maps.append(m)
    return maps


def run_on_cores(inputs, **spmd_kwargs):
    """Build (cached), run on the 8 NeuronCores, return BassKernelResults."""
    from concourse import bass_utils
    nc = _build_nc()
    return bass_utils.run_bass_kernel_spmd(
        nc, _in_maps(inputs), core_ids=list(range(NCORES)), **spmd_kwargs)


def kernel(**inputs):
    res = run_on_cores(inputs)
    out = np.empty((B, T, C), dtype=np.float32)
    for c in range(NCORES):
        b, hh = divmod(c, 2)
        out[b, :, hh * O:(hh + 1) * O] = res.results[c]["out"]
    return out
